# revision 14
# baseline (speedup 1.0000x reference)
"""Batched MoE (top-2, 8 experts) on 8 Trainium2 NeuronCores.

Strategy: expert-parallel — core e owns expert e's weights (w1/w2/w3) and
processes the tokens routed to it. Routing (sort by expert / capacity
padding) and the combine (weighting by gate prob + scatter-add over top-k)
are cheap O(tokens) index ops done on host; all matmul FLOPs run on device.

Tokens whose two top-k slots picked the SAME expert are merged into one
assignment with combine weight (w0+w1) — exact math, and it shrinks the
max per-expert column count (the SPMD capacity C every core pays for).

Device dataflow per core (capacity C columns, zero-padded):
    xt  = X_e^T               [1024, C]   (d on partitions)
    GT  = w1^T @ xt           [4096, C]   lhsT = w1 tiles (natural layout)
    VT  = w2^T @ xt           [4096, C]
    HT  = silu(GT) * VT       [4096, C]
    OT  = w3^T @ HT           [1024, C]   lhsT = w3 tiles (natural layout)

All matmul operands are bf16 (1 cycle/row on the PE, same as fp32r, but
half the HBM traffic). PSUM accumulation is fp32; the OT accumulator in
SBUF is fp32; only the final chunk's add converts to bf16 for the store.

DMA shaping: HBM descriptors are per-partition-line and pay a fixed cost,
so all weights are repacked on the host into [128, *] buffers whose
partition lines are contiguous in DRAM; each f-chunk of w1+w2 (and of w3)
moves as ONE fat dma_start with 2-16 KB lines. Concurrent DMAs complete
round-robin (≈together), so the first matmul waits on ALL initially
outstanding bytes — the leading chunks are small (128/128/256) to keep
that under ~2.3 MB. (A PE warmup burst was tried and REMOVED: pushing the
fleet to ~100% PE-busy tripped a chip-level ~2.0 GHz power downclock
that cost more than the idle it saved.) Phase B (OT accumulation) of chunk ch
is issued after phase A of chunk ch+1 so the PE never waits on the
ACT/DVE epilogue that produces HT. OT stores go out per m-pair,
interleaved with the final matmuls, to keep the drain off the tail.
"""

import numpy as np

N_EXPERTS = 8
D_MODEL = 1024
D_FF = 4096
# d_ff chunk sizes streamed through SBUF; leading chunks small so the
# initially-outstanding DMA (what the first matmul effectively waits on)
# stays small
CHUNKS = [128, 128, 256, 512, 512, 512, 512, 512, 512, 512]
assert sum(CHUNKS) == D_FF
KT = D_MODEL // 128     # 8 k-tiles (contraction d)
MT = D_MODEL // 128     # 8 output d-tiles

_program_cache = {}


def _col_chunks(C):
    """Split C columns into <=512 even pieces (PSUM bank limit for fp32 out)."""
    if C <= 512:
        return [(0, C)]
    n = (C + 511) // 512
    base = C // n
    rem = C - base * n
    out = []
    off = 0
    for i in range(n):
        sz = base + (1 if i < rem else 0)
        out.append((off, sz))
        off += sz
    return out


def _build_program(C):
    import concourse.bacc as bacc
    import concourse.mybir as mybir
    from concourse.tile import TileContext

    BF16 = mybir.dt.bfloat16
    F32 = mybir.dt.float32
    SILU = mybir.ActivationFunctionType.Silu
    ccs = _col_chunks(C)

    NCH = len(CHUNKS)
    f_offs = [sum(CHUNKS[:i]) for i in range(NCH)]
    w12_offs = [sum(16 * fc for fc in CHUNKS[:i]) for i in range(NCH)]
    w3_offs = [8 * f_offs[i] for i in range(NCH)]

    nc = bacc.Bacc()
    xt_d = nc.declare_dram_parameter("xt", [128, KT * C], BF16, isOutput=False)
    w12_d = nc.declare_dram_parameter("w12", [128, 2 * KT * D_FF], BF16, isOutput=False)
    w3_d = nc.declare_dram_parameter("w3p", [128, D_FF * 8], BF16, isOutput=False)
    ot_d = nc.declare_dram_parameter("ot", [128, MT * C], BF16, isOutput=True)

    with TileContext(nc) as tc:
        with (
            tc.tile_pool(name="xtp", bufs=1) as xt_pool,
            tc.tile_pool(name="w12", bufs=2) as w12_pool,
            tc.tile_pool(name="w3p", bufs=2) as w3_pool,
            tc.tile_pool(name="htp", bufs=2) as ht_pool,
            tc.tile_pool(name="otp", bufs=1) as ot_pool,
            tc.tile_pool(name="ot16", bufs=1) as ot16_pool,
            tc.tile_pool(name="tmp", bufs=4) as tmp_pool,
            tc.tile_pool(name="pg", bufs=2, space="PSUM") as pg_pool,
            tc.tile_pool(name="pv", bufs=2, space="PSUM") as pv_pool,
            tc.tile_pool(name="po", bufs=3, space="PSUM") as po_pool,
        ):
            xt_sb = xt_pool.tile([128, KT * C], BF16, tag="xt", name="xt")
            ot_sb = [
                ot_pool.tile([128, C], F32, tag=f"ot{m}", name=f"ot{m}")
                for m in range(MT)
            ]
            # bf16 store staging, one tile per m-pair so each store DMA has
            # 2*C-wide contiguous partition lines
            ot16_sb = [
                ot16_pool.tile([128, 2 * C], BF16, tag=f"o16{mp}", name=f"o16{mp}")
                for mp in range(MT // 2)
            ]

            cc_offs = [KT * c0 for c0, _ in ccs]

            def xcc(k, cci):
                c0, cl = ccs[cci]
                base = cc_offs[cci] + k * cl
                return xt_sb[:, base : base + cl]

            def load_xt_block(cci):
                # one cc block = [k][c] packed, split in k-pair pieces so
                # several DMA queues drain it concurrently
                c0, cl = ccs[cci]
                base = cc_offs[cci]
                for k in range(0, KT, 2):
                    nc.sync.dma_start(
                        out=xt_sb[:, base + k * cl : base + (k + 2) * cl],
                        in_=xt_d[:, base + k * cl : base + (k + 2) * cl],
                    )

            def load_w12(ch):
                fc = CHUNKS[ch]
                off = w12_offs[ch]
                t = w12_pool.tile([128, 16 * fc], BF16, tag="w12", name=f"w12c{ch}")
                if ch < 2:
                    # startup chunks: quarter pieces on separate queue rows —
                    # concurrent DMAs drain byte-fair per row, so more rows
                    # finish the startup-critical set sooner
                    for q in range(4):
                        nc.sync.dma_start(
                            out=t[:, q * 4 * fc : (q + 1) * 4 * fc],
                            in_=w12_d[:, off + q * 4 * fc : off + (q + 1) * 4 * fc],
                        )
                else:
                    nc.sync.dma_start(out=t[:], in_=w12_d[:, off : off + 16 * fc])
                return t

            def load_w3(ch):
                fc = CHUNKS[ch]
                off = w3_offs[ch]
                t = w3_pool.tile([128, 8 * fc], BF16, tag="w3", name=f"w3c{ch}")
                nc.sync.dma_start(out=t[:], in_=w3_d[:, off : off + 8 * fc])
                return t

            def phase_a(ch, w12):
                """GT/VT matmuls + silu*mul epilogue -> HT tiles for a chunk."""
                fc = CHUNKS[ch]
                jt = fc // 128
                hts = []
                for jj in range(jt):
                    ht_t = ht_pool.tile(
                        [128, C], BF16, tag=f"ht{jj % 4}", name=f"ht{jj}"
                    )
                    for cci, (c0, cl) in enumerate(ccs):
                        cs = slice(c0, c0 + cl)
                        pg = pg_pool.tile([128, cl], F32, tag="pg", name="pg")
                        pv = pv_pool.tile([128, cl], F32, tag="pv", name="pv")
                        for k in range(KT):
                            ws = slice(k * fc + jj * 128, k * fc + (jj + 1) * 128)
                            nc.tensor.matmul(
                                out=pg[:],
                                lhsT=w12[:, ws],
                                rhs=xcc(k, cci),
                                start=(k == 0),
                                stop=(k == KT - 1),
                            )
                        for k in range(KT):
                            ws = slice(
                                8 * fc + k * fc + jj * 128,
                                8 * fc + k * fc + (jj + 1) * 128,
                            )
                            nc.tensor.matmul(
                                out=pv[:],
                                lhsT=w12[:, ws],
                                rhs=xcc(k, cci),
                                start=(k == 0),
                                stop=(k == KT - 1),
                            )
                        st = tmp_pool.tile([128, cl], F32, tag="silu", name="st")
                        nc.scalar.activation(st[:], pg[:], SILU)
                        nc.vector.tensor_mul(out=ht_t[:, cs], in0=st[:], in1=pv[:])
                    hts.append(ht_t)
                return hts

            def phase_b_m(ch, w3c, hts, m):
                """OT partial accumulation for one output d-tile of a chunk."""
                jt = len(hts)
                for c0, cl in ccs:
                    cs = slice(c0, c0 + cl)
                    po = po_pool.tile([128, cl], F32, tag="po", name="po")
                    for jj in range(jt):
                        ws = slice(jj * 1024 + m * 128, jj * 1024 + (m + 1) * 128)
                        nc.tensor.matmul(
                            out=po[:],
                            lhsT=w3c[:, ws],
                            rhs=hts[jj][:, cs],
                            start=(jj == 0),
                            stop=(jj == jt - 1),
                        )
                    if ch == 0:
                        nc.vector.tensor_copy(out=ot_sb[m][:, cs], in_=po[:])
                    elif ch == NCH - 1:
                        # final accumulation converts to bf16 for the store
                        nc.vector.tensor_add(
                            out=ot16_sb[m // 2][
                                :, (m % 2) * C + c0 : (m % 2) * C + c0 + cl
                            ],
                            in0=ot_sb[m][:, cs],
                            in1=po[:],
                        )
                    else:
                        nc.vector.tensor_add(
                            out=ot_sb[m][:, cs], in0=ot_sb[m][:, cs], in1=po[:]
                        )

            def phase_b(ch, w3c, hts):
                for m in range(MT):
                    phase_b_m(ch, w3c, hts, m)

            # software pipeline: B(ch) issues after A(ch+1) so phase B never
            # stalls the PE on the ACT/DVE epilogue that produces its HT
            # input. DMAs are emitted in PE consumption order. The last two
            # B passes interleave m-wise so each OT m-pair's store DMA
            # overlaps the remaining matmuls.
            # stage 1: only what the first matmul groups need — xt cc-block
            # 0 and chunk 0's w1/w2. Everything later is gated behind the
            # tiny DVE reader below: its strided read spans the whole xt
            # tile, so the cc-block-1 DMAs (and, via SP-ring FIFO, every
            # subsequent DMA) wait until stage 1 has landed. This keeps the
            # startup DMA bandwidth on the critical bytes.
            load_xt_block(0)
            w12 = load_w12(0)
            if len(ccs) > 1:
                cl0 = ccs[0][1]
                gate = tmp_pool.tile([1, KT * len(ccs)], F32, tag="gate", name="gate")
                nc.vector.tensor_copy(
                    out=gate[:],
                    in_=xt_sb[0:1, cl0 - 1 : KT * C : cl0],
                )
                for cci in range(1, len(ccs)):
                    load_xt_block(cci)
            hts_prev = phase_a(0, w12)
            w3_prev = None
            for ch in range(1, NCH):
                w12 = load_w12(ch)
                w3_prev = load_w3(ch - 1)
                hts = phase_a(ch, w12)
                if ch < NCH - 1:
                    phase_b(ch - 1, w3_prev, hts_prev)
                    hts_prev = hts
            w3_last = load_w3(NCH - 1)
            for m in range(MT):
                phase_b_m(NCH - 2, w3_prev, hts_prev, m)
                phase_b_m(NCH - 1, w3_last, hts, m)
                if m % 2 == 1:
                    mp = m // 2
                    if mp == MT // 2 - 1:
                        # last pair: two half-stores so the tail drains sooner
                        nc.sync.dma_start(
                            out=ot_d[:, mp * 2 * C : mp * 2 * C + C],
                            in_=ot16_sb[mp][:, :C],
                        )
                        nc.sync.dma_start(
                            out=ot_d[:, mp * 2 * C + C : (mp + 1) * 2 * C],
                            in_=ot16_sb[mp][:, C:],
                        )
                    else:
                        nc.sync.dma_start(
                            out=ot_d[:, mp * 2 * C : (mp + 1) * 2 * C],
                            in_=ot16_sb[mp][:],
                        )

    nc.compile()
    return nc


def _get_program(C):
    if C not in _program_cache:
        _program_cache[C] = _build_program(C)
    return _program_cache[C]


def _run(nc, in_maps, trace=False):
    import time

    from concourse.bass_utils import run_bass_kernel_spmd

    last = None
    for attempt in range(4):
        try:
            return run_bass_kernel_spmd(
                nc, in_maps, list(range(N_EXPERTS)), trace=trace
            )
        except Exception as e:  # stale device state from a prior crashed run
            last = e
            time.sleep(10 * (attempt + 1))
            try:  # poke the runtime with a trivial op to clear/verify state
                import jax
                import jax.numpy as jnp

                jnp.add(jnp.ones((8, 8)), 1.0).block_until_ready()
            except Exception:
                pass
    raise last


def _pack_w12(w1e16, w2e16):
    """[1024,4096]x2 bf16 -> [128, 65536] packed per CHUNKS: for each chunk,
    8 w1 k-slabs then 8 w2 k-slabs, each [128, fc] with contiguous lines."""
    w1v = w1e16.reshape(KT, 128, D_FF)
    w2v = w2e16.reshape(KT, 128, D_FF)
    parts = []
    for i, fc in enumerate(CHUNKS):
        f0 = sum(CHUNKS[:i])
        parts.append(
            w1v[:, :, f0 : f0 + fc].transpose(1, 0, 2).reshape(128, KT * fc)
        )
        parts.append(
            w2v[:, :, f0 : f0 + fc].transpose(1, 0, 2).reshape(128, KT * fc)
        )
    return np.ascontiguousarray(np.concatenate(parts, axis=1))


def _pack_w3(w3e16):
    """[4096,1024] bf16 -> [128, 32768]: per chunk, jt j-slabs [128,1024]."""
    w3v = w3e16.reshape(D_FF // 128, 128, D_MODEL)
    parts = []
    for i, fc in enumerate(CHUNKS):
        j0 = sum(CHUNKS[:i]) // 128
        jt = fc // 128
        parts.append(
            w3v[j0 : j0 + jt].transpose(1, 0, 2).reshape(128, jt * D_MODEL)
        )
    return np.ascontiguousarray(np.concatenate(parts, axis=1))


def kernel(x, expert_indices, expert_weights, w1, w2, w3, _trace=False):
    import ml_dtypes

    BF16 = ml_dtypes.bfloat16

    x = np.ascontiguousarray(np.asarray(x, dtype=np.float32))
    expert_indices = np.asarray(expert_indices)
    expert_weights = np.asarray(expert_weights, dtype=np.float32)
    w1 = np.asarray(w1, dtype=np.float32)
    w2 = np.asarray(w2, dtype=np.float32)
    w3 = np.asarray(w3, dtype=np.float32)

    n_tokens, d_model = x.shape
    n_experts = w1.shape[0]

    # assignments with [e,e] top-2 duplicates merged (weight w0+w1) — exact,
    # and reduces the max per-expert count, i.e. the padded capacity C
    e0, e1 = expert_indices[:, 0].astype(np.int64), expert_indices[:, 1].astype(
        np.int64
    )
    wt0, wt1 = expert_weights[:, 0], expert_weights[:, 1]
    dup = e0 == e1
    a_tok = np.concatenate([np.arange(n_tokens), np.arange(n_tokens)[~dup]])
    a_e = np.concatenate([e0, e1[~dup]])
    a_w = np.concatenate([np.where(dup, wt0 + wt1, wt0), wt1[~dup]]).astype(
        np.float32
    )
    A = len(a_e)

    order = np.argsort(a_e, kind="stable")
    s_tok = a_tok[order]
    s_w = a_w[order]
    counts = np.bincount(a_e, minlength=n_experts)
    starts = np.concatenate([[0], np.cumsum(counts)[:-1]])

    C = int(counts.max())
    C = max(256, -(-C // 4) * 4)  # round up to multiple of 4 (8B bf16 rows)

    in_maps = []
    for e in range(n_experts):
        seg = s_tok[starts[e] : starts[e] + counts[e]]
        xte = np.zeros((KT, 128, C), BF16)
        xte[:, :, : counts[e]] = x[seg].astype(BF16).T.reshape(KT, 128, counts[e])
        # cc-block-major packing: [cc][k][c] (matches the device-side layout)
        xt_blocks = [
            xte[:, :, c0 : c0 + cl].transpose(1, 0, 2).reshape(128, KT * cl)
            for c0, cl in _col_chunks(C)
        ]
        in_maps.append(
            {
                "xt": np.ascontiguousarray(np.concatenate(xt_blocks, axis=1)),
                "w12": _pack_w12(w1[e].astype(BF16), w2[e].astype(BF16)),
                "w3p": _pack_w3(w3[e].astype(BF16)),
            }
        )

    nc = _get_program(C)
    res = _run(nc, in_maps, trace=_trace)

    y = np.empty((A, d_model), np.float32)
    for e in range(n_experts):
        ot = np.asarray(res.results[e]["ot"])  # [128, MT*C] bf16
        ot = (
            ot.reshape(128, MT, C)
            .transpose(1, 0, 2)
            .reshape(d_model, C)
            .astype(np.float32)
        )
        y[starts[e] : starts[e] + counts[e]] = ot[:, : counts[e]].T
    y *= s_w[:, None]
    y_orig = np.empty_like(y)
    y_orig[order] = y
    # assignment list = [slot0/merged for every token; slot1 for ~dup tokens]
    out = y_orig[:n_tokens].copy()
    out[~dup] += y_orig[n_tokens:]
    if _trace:
        return out.astype(np.float32, copy=False), res
    return out.astype(np.float32, copy=False)


# revision 15
# speedup vs baseline: 1.0190x; 1.0190x over previous
"""Batched MoE (top-2, 8 experts) on 8 Trainium2 NeuronCores.

Strategy: expert-parallel — core e owns expert e's weights (w1/w2/w3) and
processes the tokens routed to it. Routing (sort by expert / capacity
padding) and the combine (weighting by gate prob + scatter-add over top-k)
are cheap O(tokens) index ops done on host; all matmul FLOPs run on device.

Tokens whose two top-k slots picked the SAME expert are merged into one
assignment with combine weight (w0+w1) — exact math, and it shrinks the
max per-expert column count (the SPMD capacity C every core pays for).

Device dataflow per core (capacity C columns, zero-padded):
    xt  = X_e^T               [1024, C]   (d on partitions)
    GT  = w1^T @ xt           [4096, C]   lhsT = w1 tiles (natural layout)
    VT  = w2^T @ xt           [4096, C]
    HT  = silu(GT) * VT       [4096, C]
    OT  = w3^T @ HT           [1024, C]   lhsT = w3 tiles (natural layout)

All matmul operands are bf16 (1 cycle/row on the PE, same as fp32r, but
half the HBM traffic). PSUM accumulation is fp32; the OT accumulator in
SBUF is fp32; only the final chunk's add converts to bf16 for the store.

DMA shaping: HBM descriptors are per-partition-line and pay a fixed cost,
so all weights are repacked on the host into [128, *] buffers whose
partition lines are contiguous in DRAM; each f-chunk of w1+w2 (and of w3)
moves as ONE fat dma_start with 2-16 KB lines. Concurrent DMAs complete
round-robin (≈together), so the first matmul waits on ALL initially
outstanding bytes — the leading chunks are small (128/128/256) to keep
that under ~2.3 MB. (A PE warmup burst was tried and REMOVED: pushing the
fleet to ~100% PE-busy tripped a chip-level ~2.0 GHz power downclock
that cost more than the idle it saved.) Phase B (OT accumulation) of chunk ch
is issued after phase A of chunk ch+1 so the PE never waits on the
ACT/DVE epilogue that produces HT. OT stores go out per m-pair,
interleaved with the final matmuls, to keep the drain off the tail.
"""

import numpy as np

N_EXPERTS = 8
D_MODEL = 1024
D_FF = 4096
# d_ff chunk sizes streamed through SBUF; leading chunks small so the
# initially-outstanding DMA (what the first matmul effectively waits on)
# stays small
CHUNKS = [128, 128, 256, 512, 512, 512, 512, 512, 512, 512]
assert sum(CHUNKS) == D_FF
KT = D_MODEL // 128     # 8 k-tiles (contraction d)
MT = D_MODEL // 128     # 8 output d-tiles

_program_cache = {}


def _col_chunks(C):
    """Split C columns into <=512 even pieces (PSUM bank limit for fp32 out)."""
    if C <= 512:
        return [(0, C)]
    n = (C + 511) // 512
    base = C // n
    rem = C - base * n
    out = []
    off = 0
    for i in range(n):
        sz = base + (1 if i < rem else 0)
        out.append((off, sz))
        off += sz
    return out


def _build_program(C):
    import concourse.bacc as bacc
    import concourse.mybir as mybir
    from concourse.tile import TileContext

    BF16 = mybir.dt.bfloat16
    F32 = mybir.dt.float32
    SILU = mybir.ActivationFunctionType.Silu
    ccs = _col_chunks(C)

    NCH = len(CHUNKS)
    f_offs = [sum(CHUNKS[:i]) for i in range(NCH)]
    w12_offs = [sum(16 * fc for fc in CHUNKS[:i]) for i in range(NCH)]
    w3_offs = [8 * f_offs[i] for i in range(NCH)]

    nc = bacc.Bacc()
    xt_d = nc.declare_dram_parameter("xt", [128, KT * C], BF16, isOutput=False)
    w12_d = nc.declare_dram_parameter("w12", [128, 2 * KT * D_FF], BF16, isOutput=False)
    w3_d = nc.declare_dram_parameter("w3p", [128, D_FF * 8], BF16, isOutput=False)
    ot_d = nc.declare_dram_parameter("ot", [128, MT * C], BF16, isOutput=True)

    with TileContext(nc) as tc:
        with (
            tc.tile_pool(name="xtp", bufs=1) as xt_pool,
            tc.tile_pool(name="w12", bufs=2) as w12_pool,
            tc.tile_pool(name="w3p", bufs=2) as w3_pool,
            tc.tile_pool(name="htp", bufs=2) as ht_pool,
            tc.tile_pool(name="otp", bufs=1) as ot_pool,
            tc.tile_pool(name="ot16", bufs=1) as ot16_pool,
            tc.tile_pool(name="tmp", bufs=4) as tmp_pool,
            tc.tile_pool(name="pg", bufs=2, space="PSUM") as pg_pool,
            tc.tile_pool(name="pv", bufs=2, space="PSUM") as pv_pool,
            tc.tile_pool(name="po", bufs=3, space="PSUM") as po_pool,
        ):
            xt_sb = xt_pool.tile([128, KT * C], BF16, tag="xt", name="xt")
            ot_sb = [
                ot_pool.tile([128, C], F32, tag=f"ot{m}", name=f"ot{m}")
                for m in range(MT)
            ]
            # bf16 store staging, one tile per m-pair so each store DMA has
            # 2*C-wide contiguous partition lines
            ot16_sb = [
                ot16_pool.tile([128, 2 * C], BF16, tag=f"o16{mp}", name=f"o16{mp}")
                for mp in range(MT // 2)
            ]

            cc_offs = [KT * c0 for c0, _ in ccs]

            def xcc(k, cci):
                c0, cl = ccs[cci]
                base = cc_offs[cci] + k * cl
                return xt_sb[:, base : base + cl]

            def load_xt_block(cci):
                # one cc block = [k][c] packed, split in k-pair pieces so
                # several DMA queues drain it concurrently
                c0, cl = ccs[cci]
                base = cc_offs[cci]
                for k in range(0, KT, 2):
                    nc.sync.dma_start(
                        out=xt_sb[:, base + k * cl : base + (k + 2) * cl],
                        in_=xt_d[:, base + k * cl : base + (k + 2) * cl],
                    )

            def load_w12(ch):
                fc = CHUNKS[ch]
                off = w12_offs[ch]
                t = w12_pool.tile([128, 16 * fc], BF16, tag="w12", name=f"w12c{ch}")
                if ch < 2:
                    # startup chunks: w1 half then w2 half on separate queues
                    nc.sync.dma_start(
                        out=t[:, : 8 * fc], in_=w12_d[:, off : off + 8 * fc]
                    )
                    nc.sync.dma_start(
                        out=t[:, 8 * fc :],
                        in_=w12_d[:, off + 8 * fc : off + 16 * fc],
                    )
                else:
                    nc.sync.dma_start(out=t[:], in_=w12_d[:, off : off + 16 * fc])
                return t

            def load_w3(ch):
                fc = CHUNKS[ch]
                off = w3_offs[ch]
                t = w3_pool.tile([128, 8 * fc], BF16, tag="w3", name=f"w3c{ch}")
                nc.sync.dma_start(out=t[:], in_=w3_d[:, off : off + 8 * fc])
                return t

            def phase_a(ch, w12):
                """GT/VT matmuls + silu*mul epilogue -> HT tiles for a chunk."""
                fc = CHUNKS[ch]
                jt = fc // 128
                hts = []
                for jj in range(jt):
                    ht_t = ht_pool.tile(
                        [128, C], BF16, tag=f"ht{jj % 4}", name=f"ht{jj}"
                    )
                    for cci, (c0, cl) in enumerate(ccs):
                        cs = slice(c0, c0 + cl)
                        pg = pg_pool.tile([128, cl], F32, tag="pg", name="pg")
                        pv = pv_pool.tile([128, cl], F32, tag="pv", name="pv")
                        for k in range(KT):
                            ws = slice(k * fc + jj * 128, k * fc + (jj + 1) * 128)
                            nc.tensor.matmul(
                                out=pg[:],
                                lhsT=w12[:, ws],
                                rhs=xcc(k, cci),
                                start=(k == 0),
                                stop=(k == KT - 1),
                            )
                        for k in range(KT):
                            ws = slice(
                                8 * fc + k * fc + jj * 128,
                                8 * fc + k * fc + (jj + 1) * 128,
                            )
                            nc.tensor.matmul(
                                out=pv[:],
                                lhsT=w12[:, ws],
                                rhs=xcc(k, cci),
                                start=(k == 0),
                                stop=(k == KT - 1),
                            )
                        st = tmp_pool.tile([128, cl], F32, tag="silu", name="st")
                        nc.scalar.activation(st[:], pg[:], SILU)
                        nc.vector.tensor_mul(out=ht_t[:, cs], in0=st[:], in1=pv[:])
                    hts.append(ht_t)
                return hts

            def phase_b_m(ch, w3c, hts, m):
                """OT partial accumulation for one output d-tile of a chunk."""
                jt = len(hts)
                for c0, cl in ccs:
                    cs = slice(c0, c0 + cl)
                    po = po_pool.tile([128, cl], F32, tag="po", name="po")
                    for jj in range(jt):
                        ws = slice(jj * 1024 + m * 128, jj * 1024 + (m + 1) * 128)
                        nc.tensor.matmul(
                            out=po[:],
                            lhsT=w3c[:, ws],
                            rhs=hts[jj][:, cs],
                            start=(jj == 0),
                            stop=(jj == jt - 1),
                        )
                    if ch == 0:
                        nc.vector.tensor_copy(out=ot_sb[m][:, cs], in_=po[:])
                    elif ch == NCH - 1:
                        # final accumulation converts to bf16 for the store
                        nc.vector.tensor_add(
                            out=ot16_sb[m // 2][
                                :, (m % 2) * C + c0 : (m % 2) * C + c0 + cl
                            ],
                            in0=ot_sb[m][:, cs],
                            in1=po[:],
                        )
                    else:
                        nc.vector.tensor_add(
                            out=ot_sb[m][:, cs], in0=ot_sb[m][:, cs], in1=po[:]
                        )

            def phase_b(ch, w3c, hts):
                for m in range(MT):
                    phase_b_m(ch, w3c, hts, m)

            # software pipeline: B(ch) issues after A(ch+1) so phase B never
            # stalls the PE on the ACT/DVE epilogue that produces its HT
            # input. DMAs are emitted in PE consumption order. The last two
            # B passes interleave m-wise so each OT m-pair's store DMA
            # overlaps the remaining matmuls.
            # stage 1: only what the first matmul groups need — xt cc-block
            # 0 and chunk 0's w1/w2. Everything later is gated behind the
            # tiny DVE reader below: its strided read spans the whole xt
            # tile, so the cc-block-1 DMAs (and, via SP-ring FIFO, every
            # subsequent DMA) wait until stage 1 has landed. This keeps the
            # startup DMA bandwidth on the critical bytes.
            load_xt_block(0)
            w12 = load_w12(0)
            if len(ccs) > 1:
                cl0 = ccs[0][1]
                gate = tmp_pool.tile([1, KT * len(ccs)], F32, tag="gate", name="gate")
                nc.vector.tensor_copy(
                    out=gate[:],
                    in_=xt_sb[0:1, cl0 - 1 : KT * C : cl0],
                )
                for cci in range(1, len(ccs)):
                    load_xt_block(cci)
            hts_prev = phase_a(0, w12)
            w3_prev = None
            for ch in range(1, NCH):
                w12 = load_w12(ch)
                w3_prev = load_w3(ch - 1)
                hts = phase_a(ch, w12)
                if ch < NCH - 1:
                    phase_b(ch - 1, w3_prev, hts_prev)
                    hts_prev = hts
            w3_last = load_w3(NCH - 1)
            for m in range(MT):
                phase_b_m(NCH - 2, w3_prev, hts_prev, m)
                phase_b_m(NCH - 1, w3_last, hts, m)
                if m % 2 == 1:
                    mp = m // 2
                    nc.sync.dma_start(
                        out=ot_d[:, mp * 2 * C : (mp + 1) * 2 * C],
                        in_=ot16_sb[mp][:],
                    )

    nc.compile()
    return nc


def _get_program(C):
    if C not in _program_cache:
        _program_cache[C] = _build_program(C)
    return _program_cache[C]


def _run(nc, in_maps, trace=False):
    import time

    from concourse.bass_utils import run_bass_kernel_spmd

    last = None
    for attempt in range(4):
        try:
            return run_bass_kernel_spmd(
                nc, in_maps, list(range(N_EXPERTS)), trace=trace
            )
        except Exception as e:  # stale device state from a prior crashed run
            last = e
            time.sleep(10 * (attempt + 1))
            try:  # poke the runtime with a trivial op to clear/verify state
                import jax
                import jax.numpy as jnp

                jnp.add(jnp.ones((8, 8)), 1.0).block_until_ready()
            except Exception:
                pass
    raise last


def _pack_w12(w1e16, w2e16):
    """[1024,4096]x2 bf16 -> [128, 65536] packed per CHUNKS: for each chunk,
    8 w1 k-slabs then 8 w2 k-slabs, each [128, fc] with contiguous lines."""
    w1v = w1e16.reshape(KT, 128, D_FF)
    w2v = w2e16.reshape(KT, 128, D_FF)
    parts = []
    for i, fc in enumerate(CHUNKS):
        f0 = sum(CHUNKS[:i])
        parts.append(
            w1v[:, :, f0 : f0 + fc].transpose(1, 0, 2).reshape(128, KT * fc)
        )
        parts.append(
            w2v[:, :, f0 : f0 + fc].transpose(1, 0, 2).reshape(128, KT * fc)
        )
    return np.ascontiguousarray(np.concatenate(parts, axis=1))


def _pack_w3(w3e16):
    """[4096,1024] bf16 -> [128, 32768]: per chunk, jt j-slabs [128,1024]."""
    w3v = w3e16.reshape(D_FF // 128, 128, D_MODEL)
    parts = []
    for i, fc in enumerate(CHUNKS):
        j0 = sum(CHUNKS[:i]) // 128
        jt = fc // 128
        parts.append(
            w3v[j0 : j0 + jt].transpose(1, 0, 2).reshape(128, jt * D_MODEL)
        )
    return np.ascontiguousarray(np.concatenate(parts, axis=1))


def kernel(x, expert_indices, expert_weights, w1, w2, w3, _trace=False):
    import ml_dtypes

    BF16 = ml_dtypes.bfloat16

    x = np.ascontiguousarray(np.asarray(x, dtype=np.float32))
    expert_indices = np.asarray(expert_indices)
    expert_weights = np.asarray(expert_weights, dtype=np.float32)
    w1 = np.asarray(w1, dtype=np.float32)
    w2 = np.asarray(w2, dtype=np.float32)
    w3 = np.asarray(w3, dtype=np.float32)

    n_tokens, d_model = x.shape
    n_experts = w1.shape[0]

    # assignments with [e,e] top-2 duplicates merged (weight w0+w1) — exact,
    # and reduces the max per-expert count, i.e. the padded capacity C
    e0, e1 = expert_indices[:, 0].astype(np.int64), expert_indices[:, 1].astype(
        np.int64
    )
    wt0, wt1 = expert_weights[:, 0], expert_weights[:, 1]
    dup = e0 == e1
    a_tok = np.concatenate([np.arange(n_tokens), np.arange(n_tokens)[~dup]])
    a_e = np.concatenate([e0, e1[~dup]])
    a_w = np.concatenate([np.where(dup, wt0 + wt1, wt0), wt1[~dup]]).astype(
        np.float32
    )
    A = len(a_e)

    order = np.argsort(a_e, kind="stable")
    s_tok = a_tok[order]
    s_w = a_w[order]
    counts = np.bincount(a_e, minlength=n_experts)
    starts = np.concatenate([[0], np.cumsum(counts)[:-1]])

    C = int(counts.max())
    C = max(256, -(-C // 4) * 4)  # round up to multiple of 4 (8B bf16 rows)

    in_maps = []
    for e in range(n_experts):
        seg = s_tok[starts[e] : starts[e] + counts[e]]
        xte = np.zeros((KT, 128, C), BF16)
        xte[:, :, : counts[e]] = x[seg].astype(BF16).T.reshape(KT, 128, counts[e])
        # cc-block-major packing: [cc][k][c] (matches the device-side layout)
        xt_blocks = [
            xte[:, :, c0 : c0 + cl].transpose(1, 0, 2).reshape(128, KT * cl)
            for c0, cl in _col_chunks(C)
        ]
        in_maps.append(
            {
                "xt": np.ascontiguousarray(np.concatenate(xt_blocks, axis=1)),
                "w12": _pack_w12(w1[e].astype(BF16), w2[e].astype(BF16)),
                "w3p": _pack_w3(w3[e].astype(BF16)),
            }
        )

    nc = _get_program(C)
    res = _run(nc, in_maps, trace=_trace)

    y = np.empty((A, d_model), np.float32)
    for e in range(n_experts):
        ot = np.asarray(res.results[e]["ot"])  # [128, MT*C] bf16
        ot = (
            ot.reshape(128, MT, C)
            .transpose(1, 0, 2)
            .reshape(d_model, C)
            .astype(np.float32)
        )
        y[starts[e] : starts[e] + counts[e]] = ot[:, : counts[e]].T
    y *= s_w[:, None]
    y_orig = np.empty_like(y)
    y_orig[order] = y
    # assignment list = [slot0/merged for every token; slot1 for ~dup tokens]
    out = y_orig[:n_tokens].copy()
    out[~dup] += y_orig[n_tokens:]
    if _trace:
        return out.astype(np.float32, copy=False), res
    return out.astype(np.float32, copy=False)
